# revision 1
# baseline (speedup 1.0000x reference)
"""Trainium2 Bass kernel for the analog-crossbar CustomLayer.

Math (per 512x512 weight tile, per reference.py):
    cond   = (w - wmin)*s + G_MIN ; quantize to 16 levels
    g_eff  = 1/(1/cond + r_wire)          (Jeong nonlinear IV model)
    cur    = x @ g_eff ; ideal = x @ cond
    out    = ((cur - mean(cur))*coeff + mean(ideal) - offset)/s , coeff from
             per-row ranges of ideal/cur; summed over in_tiles, plus bias.

Sharding: out_tiles (columns of weight) across 8 cores; x replicated.
Each core computes a [1024, 512] slice; host concatenates.

Device mapping highlights:
  - matmuls in float32r (FP22 truncation, full PE speed at N=512)
  - ideal matmul runs against the exact integer quantization levels (fp8e4,
    exact for 0..15), reconstructed as ideal = step*ideal' + G_MIN*rowsum
  - round() via the +-1.5*2^23 trick on tensor_scalar (round-half-even,
    matches jnp.round)
  - reciprocals via Ln/Exp on ScalarE (g = q * exp(-ln(1 + q*r)))
  - per-row sums via ScalarE activation accum_out; max/min via VectorE
    reduces; in_tile accumulation via PE identity-matmul into PSUM
"""

import numpy as np
import sys

sys.path.insert(0, "/opt/trn_rl_repo")

# ---- problem constants (hardcoded; must match reference) ----
R_HRS = 1.0e6
R_LRS = 1.0e4
RP = 2.0
BITS = 4
TS = 512
G_MIN = np.float32(1.0 / R_HRS)
G_MAX = np.float32(1.0 / R_LRS)
B = 1024          # batch
IN_F = 4096       # in features
OUT_F = 4096      # out features
NCORES = 8
IT = IN_F // TS   # 8 in tiles
KC = TS // 128    # 4 k-chunks per tile
MB = B // 128     # 8 batch chunks
C_MAGIC = 12582912.0  # 1.5 * 2**23, round-to-nearest-even trick

_CACHE = {}


def _build():
    import concourse.bass as bass
    import concourse.tile as tile
    from concourse import bacc, mybir

    f32 = mybir.dt.float32
    f32r = mybir.dt.float32r
    f8 = mybir.dt.float8e4
    Alu = mybir.AluOpType
    Act = mybir.ActivationFunctionType

    nc = bacc.Bacc(None, target_bir_lowering=False, debug=False)

    xt_d = nc.dram_tensor("xt", [IN_F, B], f32, kind="ExternalInput")
    w_d = nc.dram_tensor("w", [IN_F, TS], f32, kind="ExternalInput")
    rw_d = nc.dram_tensor("rwire", [128, KC * TS], f32, kind="ExternalInput")
    scal_d = nc.dram_tensor("scal", [128, 5 * IT], f32, kind="ExternalInput")
    rsum_d = nc.dram_tensor("rsum", [MB, 128, IT], f32, kind="ExternalInput")
    biasb_d = nc.dram_tensor("biasb", [128, TS], f32, kind="ExternalInput")
    id_d = nc.dram_tensor("ident", [128, 128], f32, kind="ExternalInput")
    out_d = nc.dram_tensor("out", [B, TS], f32, kind="ExternalOutput")

    # w rows (t c p) -> per tile t: [128, c, o] chunk layout
    w_r = w_d.ap().rearrange("(t c p) o -> t p c o", t=IT, c=KC, p=128)
    # xT rows (c p) -> [128, chunk, m-col]
    xt_r = xt_d.ap().rearrange("(c p) m -> p c m", p=128)

    with tile.TileContext(nc) as tc:
        with (
            tc.tile_pool(name="const", bufs=1) as constp,
            tc.tile_pool(name="gq", bufs=1) as gqp,
            tc.tile_pool(name="wstage", bufs=1) as wstagep,
            tc.tile_pool(name="wscratch", bufs=1) as wscr,
            tc.tile_pool(name="xm", bufs=1) as xmp,
            tc.tile_pool(name="curbuf", bufs=1) as curp,
            tc.tile_pool(name="idsc", bufs=3) as idscp,
            tc.tile_pool(name="tsc", bufs=3) as tscp,
            tc.tile_pool(name="stats", bufs=2) as statp,
            tc.tile_pool(name="outsb", bufs=2) as outp,
            tc.tile_pool(name="psA", bufs=2, space=bass.MemorySpace.PSUM) as psA,
            tc.tile_pool(name="psB", bufs=2, space=bass.MemorySpace.PSUM) as psB,
            tc.tile_pool(name="psO", bufs=2, space=bass.MemorySpace.PSUM) as psO,
        ):
            # ---- constants ----
            rw_sb = constp.tile([128, KC * TS], f32)
            nc.sync.dma_start(out=rw_sb[:], in_=rw_d.ap()[:])
            scal_sb = constp.tile([128, 5 * IT], f32)
            nc.sync.dma_start(out=scal_sb[:], in_=scal_d.ap()[:])
            biasb_sb = constp.tile([128, TS], f32)
            nc.sync.dma_start(out=biasb_sb[:], in_=biasb_d.ap()[:])
            id_sb = constp.tile([128, 128], f32r)
            nc.gpsimd.dma_start(out=id_sb[:], in_=id_d.ap()[:])

            g_all = gqp.tile([128, IT * KC * TS], f32r)    # g_eff, chunk layout
            q_all = gqp.tile([128, IT * KC * TS], f32r)    # quantized conductances

            def wmin_s(it):  # broadcast per-tile scalars (slot 4)
                return scal_sb[:, 4 * IT + it:4 * IT + it + 1]

            def a_s(it):
                return scal_sb[:, IT + it:IT + it + 1]

            def stepinvs_s(it):
                return scal_sb[:, 2 * IT + it:2 * IT + it + 1]

            def stepinvs512_s(it):
                return scal_sb[:, 3 * IT + it:3 * IT + it + 1]

            STEP = float(np.float32(G_MAX - G_MIN) / np.float32(2 ** BITS - 1))

            # ================= Phase W: weight tile -> g_eff, qlev ==========
            for it in range(IT):
                wt3 = wstagep.tile([128, KC, TS], f32, tag="wt")
                nc.sync.dma_start(out=wt3[:], in_=w_r[it])
                wt = wt3[:].rearrange("p c o -> p (c o)")

                sa = wscr.tile([128, KC * TS], f32, tag="wsA")
                qsl = q_all[:, it * KC * TS:(it + 1) * KC * TS]
                gsl = g_all[:, it * KC * TS:(it + 1) * KC * TS]

                # t1 = (w - wmin) * (s/step)
                nc.vector.tensor_scalar(out=sa[:], in0=wt,
                                        scalar1=wmin_s(it), scalar2=a_s(it),
                                        op0=Alu.subtract, op1=Alu.mult)
                # rlev = round(t1)  (round-half-even via magic constant)
                nc.vector.tensor_scalar(out=sa[:], in0=sa[:],
                                        scalar1=C_MAGIC, scalar2=-C_MAGIC,
                                        op0=Alu.add, op1=Alu.add)
                # q = rlev*step + G_MIN  (persistent)
                nc.vector.tensor_scalar(out=qsl, in0=sa[:],
                                        scalar1=STEP, scalar2=float(G_MIN),
                                        op0=Alu.mult, op1=Alu.add)
                # qr = q * r_wire
                nc.vector.tensor_tensor(out=sa[:], in0=qsl, in1=rw_sb[:],
                                        op=Alu.mult)
                # ln(1 + qr), then exp(-ln) on ScalarE
                nc.scalar.activation(sa[:], sa[:], Act.Ln, bias=1.0, scale=1.0)
                nc.scalar.activation(sa[:], sa[:], Act.Exp, bias=0.0, scale=-1.0)
                # g = q * exp(-ln(1+qr)) = 1/(1/q + r)
                nc.vector.tensor_tensor(out=gsl, in0=qsl, in1=sa[:], op=Alu.mult)

            # ================= Phase X: batch chunks ========================
            for m in range(MB):
                xm = xmp.tile([128, IT * KC, 128], f32r, tag="xm")
                nc.gpsimd.dma_start(out=xm[:], in_=xt_r[:, :, m * 128:(m + 1) * 128])
                rs = statp.tile([128, IT], f32, tag="rs")
                nc.sync.dma_start(out=rs[:], in_=rsum_d.ap()[m])

                curbuf = curp.tile([128, IT * TS], f32, tag="cur")
                cmaxb = statp.tile([128, IT], f32, tag="cmax")
                cminb = statp.tile([128, IT], f32, tag="cmin")
                imaxb = statp.tile([128, IT], f32, tag="imax")
                iminb = statp.tile([128, IT], f32, tag="imin")
                csumb = statp.tile([128, IT], f32, tag="csum")
                isumb = statp.tile([128, IT], f32, tag="isum")

                for it in range(IT):
                    cur_ps = psA.tile([128, TS], f32, tag="cur_ps")
                    id_ps = psB.tile([128, TS], f32, tag="id_ps")
                    for k in range(KC):
                        lhs = xm[:, it * KC + k, :]
                        nc.tensor.matmul(
                            cur_ps[:], lhs,
                            g_all[:, (it * KC + k) * TS:(it * KC + k + 1) * TS],
                            start=(k == 0), stop=(k == KC - 1))
                    for k in range(KC):
                        lhs = xm[:, it * KC + k, :]
                        nc.tensor.matmul(
                            id_ps[:], lhs,
                            q_all[:, (it * KC + k) * TS:(it * KC + k + 1) * TS],
                            start=(k == 0), stop=(k == KC - 1))

                    # drain + row sums on ScalarE
                    cslice = curbuf[:, it * TS:(it + 1) * TS]
                    nc.scalar.activation(cslice, cur_ps[:], Act.Identity,
                                         bias=0.0, scale=1.0,
                                         accum_out=csumb[:, it:it + 1])
                    idsc = idscp.tile([128, TS], f32, tag="idsc")
                    nc.scalar.activation(idsc[:], id_ps[:], Act.Identity,
                                         bias=0.0, scale=1.0,
                                         accum_out=isumb[:, it:it + 1])
                    # per-row max/min on VectorE
                    nc.vector.tensor_reduce(cmaxb[:, it:it + 1], cslice,
                                            axis=mybir.AxisListType.X, op=Alu.max)
                    nc.vector.tensor_reduce(cminb[:, it:it + 1], cslice,
                                            axis=mybir.AxisListType.X, op=Alu.min)
                    nc.vector.tensor_reduce(imaxb[:, it:it + 1], idsc[:],
                                            axis=mybir.AxisListType.X, op=Alu.max)
                    nc.vector.tensor_reduce(iminb[:, it:it + 1], idsc[:],
                                            axis=mybir.AxisListType.X, op=Alu.min)

                # ---- batched per-row coefficients over [128, IT] ----
                di = statp.tile([128, IT], f32, tag="di")
                dc = statp.tile([128, IT], f32, tag="dc")
                co = statp.tile([128, IT], f32, tag="co")
                Ab = statp.tile([128, IT], f32, tag="Ab")
                t1 = statp.tile([128, IT], f32, tag="t1")
                t2 = statp.tile([128, IT], f32, tag="t2")
                t3 = statp.tile([128, IT], f32, tag="t3")
                Db = statp.tile([128, IT], f32, tag="Db")

                nc.vector.tensor_tensor(out=di[:], in0=imaxb[:], in1=iminb[:],
                                        op=Alu.subtract)
                # dc = (cmax + 1e-8) - cmin
                nc.vector.scalar_tensor_tensor(out=dc[:], in0=cmaxb[:],
                                               scalar=1e-8, in1=cminb[:],
                                               op0=Alu.add, op1=Alu.subtract)
                nc.vector.reciprocal(out=dc[:], in_=dc[:])
                nc.vector.tensor_tensor(out=co[:], in0=di[:], in1=dc[:],
                                        op=Alu.mult)
                # A = coeff0 * step/s ; scal columns broadcast per tile
                nc.vector.tensor_tensor(out=Ab[:], in0=co[:],
                                        in1=scal_sb[:, 2 * IT:3 * IT], op=Alu.mult)
                # D = isum'*step/(512 s) + rsum*wmin - csum*step/(512 s)*coeff0
                nc.vector.tensor_tensor(out=t1[:], in0=isumb[:],
                                        in1=scal_sb[:, 3 * IT:4 * IT], op=Alu.mult)
                nc.vector.tensor_tensor(out=t2[:], in0=rs[:],
                                        in1=scal_sb[:, 0:IT], op=Alu.mult)
                nc.vector.tensor_tensor(out=t3[:], in0=csumb[:],
                                        in1=scal_sb[:, 3 * IT:4 * IT], op=Alu.mult)
                nc.vector.tensor_tensor(out=t3[:], in0=t3[:], in1=co[:],
                                        op=Alu.mult)
                nc.vector.tensor_tensor(out=Db[:], in0=t1[:], in1=t2[:],
                                        op=Alu.subtract)
                nc.vector.tensor_tensor(out=Db[:], in0=Db[:], in1=t3[:],
                                        op=Alu.subtract)

                # ---- scale pass + accumulate over it via PE ----
                out_ps = psO.tile([128, TS], f32, tag="out_ps")
                for it in range(IT):
                    tsc = tscp.tile([128, TS], f32r, tag="tsc")
                    nc.scalar.activation(tsc[:], curbuf[:, it * TS:(it + 1) * TS],
                                         Act.Identity,
                                         bias=Db[:, it:it + 1],
                                         scale=Ab[:, it:it + 1])
                    nc.tensor.matmul(out_ps[:], id_sb[:],
                                     tsc[:],
                                     start=(it == 0), stop=(it == IT - 1))

                osb = outp.tile([128, TS], f32, tag="osb")
                nc.vector.tensor_tensor(out=osb[:], in0=out_ps[:],
                                        in1=biasb_sb[:], op=Alu.add)
                nc.sync.dma_start(out=out_d.ap()[m * 128:(m + 1) * 128, :],
                                  in_=osb[:])

    nc.compile()
    return nc


def _host_prep(x, weight, bias):
    """Build per-core input maps. All scalar math in float32."""
    x = np.ascontiguousarray(x, dtype=np.float32)
    weight = np.ascontiguousarray(weight, dtype=np.float32)
    bias = np.ascontiguousarray(bias, dtype=np.float32)

    xt = np.ascontiguousarray(x.T)                      # [4096, 1024]
    rsum = x.reshape(B, IT, TS).sum(axis=2, dtype=np.float32)  # [1024, 8]
    rsum_r = np.ascontiguousarray(
        rsum.reshape(MB, 128, IT), dtype=np.float32)    # [m, p, it]

    wr = weight.reshape(IT, TS, NCORES, TS)
    wmin = wr.min(axis=(1, 3))                          # [it, d] f32
    wmax = wr.max(axis=(1, 3))
    gr = np.float32(G_MAX) - np.float32(G_MIN)
    s = (gr / (wmax - wmin + np.float32(1e-12))).astype(np.float32)
    step = np.float32(gr / np.float32(2 ** BITS - 1))
    a = (s / step).astype(np.float32)
    invs = (np.float32(1.0) / s).astype(np.float32)
    invs512 = (invs / np.float32(512.0)).astype(np.float32)
    goff = (np.float32(G_MIN) * invs - wmin).astype(np.float32)

    # r_wire in chunk layout [128, 4*512]
    i_glob = (np.arange(KC)[:, None, None] * 128 +
              np.arange(128)[None, :, None]).astype(np.float32)
    j = np.arange(TS, dtype=np.float32)[None, None, :]
    rw = (np.float32(RP) * ((np.float32(TS) - i_glob) + (j + np.float32(1.0))))
    rw = np.ascontiguousarray(
        rw.transpose(1, 0, 2).reshape(128, KC * TS), dtype=np.float32)

    ident = np.eye(128, dtype=np.float32)

    in_maps = []
    for d in range(NCORES):
        scal = np.empty((128, 5 * IT), dtype=np.float32)
        scal[:, 0:IT] = goff[:, d][None, :]
        scal[:, IT:2 * IT] = a[:, d][None, :]
        scal[:, 2 * IT:3 * IT] = invs[:, d][None, :]
        scal[:, 3 * IT:4 * IT] = invs512[:, d][None, :]
        scal[:, 4 * IT:5 * IT] = wmin[:, d][None, :]
        in_maps.append({
            "xt": xt,
            "w": np.ascontiguousarray(weight[:, d * TS:(d + 1) * TS]),
            "rwire": rw,
            "scal": scal,
            "rsum": rsum_r,
            "biasb": np.ascontiguousarray(
                np.broadcast_to(bias[d * TS:(d + 1) * TS], (128, TS))),
            "ident": ident,
        })
    return in_maps


def get_nc():
    if "nc" not in _CACHE:
        _CACHE["nc"] = _build()
    return _CACHE["nc"]


def kernel(x, weight, bias):
    from concourse.bass_utils import run_bass_kernel_spmd

    nc = get_nc()
    in_maps = _host_prep(x, weight, bias)
    res = run_bass_kernel_spmd(nc, in_maps, core_ids=list(range(NCORES)))
    out = np.empty((B, OUT_F), dtype=np.float32)
    for d in range(NCORES):
        out[:, d * TS:(d + 1) * TS] = res.results[d]["out"]
    return out



# revision 20
# speedup vs baseline: 2.0989x; 2.0989x over previous
"""Trainium2 Bass kernel for the analog-crossbar CustomLayer (v2).

Math per 512x512 weight tile (see reference):
    L = round((w-wmin)*s/step)            integer levels 0..15
    q = G_MIN + step*L ; g = 1/(1/q + r_wire) ; delta = q - g
    ideal = x@q ; cur = x@g = ideal - x@delta
    out  = (cur - rowmean(cur))*coeff + rowmean(ideal) - offset, coeff =
           rowrange(ideal)/rowrange(cur); summed over in_tiles, /s, + bias.

Device formulation (per core: one 512-col output slice, out_tiles sharded):
    P  = x@L          (PSUM, fp8 DoubleRow matmuls; x split x = xh + xl fp8)
    u' = x@L - x@(delta/step)   (PSUM accumulation, extra fp8 DR matmuls)
    cur = step*u' + G_MIN*xsum ; ideal = step*P + G_MIN*xsum
    coeff = (Pmax-Pmin)/(umax-umin)      per-row exact reduces
    contribution = A*u'_j + R,  A = coeff*step/s,
    R = E - A*umean  (E=(imean-offset)/s, umean/E/sums precomputed on host)
    out_row = sum_it A_it*u'_it + D + bias,  D = sum_it R_it
    The sum over it runs on the PE: lhsT=diag(A_it) fp16 matmuls into PSUM,
    plus a K=1 rank-1 matmul adding the column bias.

Engine placement: PE all matmuls; Act u'-drains (fp16), diag builds, final
drain (adds D via per-partition bias); DVE the 4 exact range reduces + small
coeff algebra; Pool pre-folds u' stats (2 level tensor-tensor max/min) to cut
DVE reduce width from 512 to 128.

Host precomputes L/delta in fp8, x in two fp8 terms, and all row sums
(O(N^2) elementwise work only; matmuls and range stats stay on device).
"""

import numpy as np
import sys

sys.path.insert(0, "/opt/trn_rl_repo")

# ---- problem constants (hardcoded; must match reference) ----
R_HRS = 1.0e6
R_LRS = 1.0e4
RP = 2.0
BITS = 4
TS = 512
G_MIN = np.float32(1.0 / R_HRS)
G_MAX = np.float32(1.0 / R_LRS)
B = 1024          # batch
IN_F = 4096       # in features
OUT_F = 4096      # out features
NCORES = 8
IT = IN_F // TS   # 8 in tiles
MB = B // 128     # 8 batch chunks
STEP = float((G_MAX - G_MIN) / np.float32(2 ** BITS - 1))

_CACHE = {}


def _build():
    import concourse.bass as bass
    import concourse.tile as tile
    from concourse import bacc, mybir

    f32 = mybir.dt.float32
    f16 = mybir.dt.float16
    f8 = mybir.dt.float8e4
    Alu = mybir.AluOpType
    Act = mybir.ActivationFunctionType
    PM = mybir.MatmulPerfMode
    AX = mybir.AxisListType.X

    nc = bacc.Bacc(None, target_bir_lowering=False, debug=False)

    # DRAM tensors
    xh_d = nc.dram_tensor("xh", [128, MB, IT * 2, 2, 128], f8, kind="ExternalInput")
    xl_d = nc.dram_tensor("xl", [128, MB, IT * 2, 2, 128], f8, kind="ExternalInput")
    l8_d = nc.dram_tensor("l8", [128, IT * 2, 2, TS], f8, kind="ExternalInput")
    nd8_d = nc.dram_tensor("nd8", [128, IT * 2, 2, TS], f8, kind="ExternalInput")
    nd8l_d = nc.dram_tensor("nd8l", [128, IT * 2, 2, TS], f8, kind="ExternalInput")
    aux_d = nc.dram_tensor("aux", [MB, 128, 3 * IT], f32, kind="ExternalInput")
    id_d = nc.dram_tensor("ident", [128, 128], f16, kind="ExternalInput")
    bias_d = nc.dram_tensor("biasr", [1, TS], f16, kind="ExternalInput")
    out_d = nc.dram_tensor("out", [B, TS], f32, kind="ExternalOutput")

    with tile.TileContext(nc) as tc:
        with (
            tc.tile_pool(name="const", bufs=1) as constp,
            tc.tile_pool(name="xm", bufs=2) as xmp,
            tc.tile_pool(name="ubuf", bufs=2) as ubufp,
            tc.tile_pool(name="stats", bufs=2) as statp,
            tc.tile_pool(name="adiag", bufs=2) as adp,
            tc.tile_pool(name="outsb", bufs=2) as outp,
            tc.tile_pool(name="psP", bufs=2, space=bass.MemorySpace.PSUM) as psP,
            tc.tile_pool(name="psU", bufs=2, space=bass.MemorySpace.PSUM) as psU,
            tc.tile_pool(name="psO", bufs=2, space=bass.MemorySpace.PSUM) as psO,
        ):
            # ---- constants: interleaved with m0 loads in first-needed order
            # so it=0 compute starts early and SP never parks on far-future
            # chunks ----
            l8_sb = constp.tile([128, IT * 2, 2, TS], f8)
            nd8_sb = constp.tile([128, IT * 2, 2, TS], f8)
            nd8l_sb = constp.tile([128, IT * 2, 2, TS], f8)
            xh0_sb = xmp.tile([128, IT * 2, 2, 128], f8, tag="xh")
            xl0_sb = xmp.tile([128, IT * 2, 2, 128], f8, tag="xl")
            aux0_sb = xmp.tile([128, 3 * IT], f32, tag="aux")
            for it in range(IT):
                sl = slice(2 * it, 2 * it + 2)
                nc.sync.dma_start(out=l8_sb[:, sl], in_=l8_d.ap()[:, sl])
                nc.sync.dma_start(out=nd8_sb[:, sl], in_=nd8_d.ap()[:, sl])
                nc.sync.dma_start(out=nd8l_sb[:, sl], in_=nd8l_d.ap()[:, sl])
                if it == 0:
                    nc.sync.dma_start(out=xh0_sb[:], in_=xh_d.ap()[:, 0])
                    nc.sync.dma_start(out=xl0_sb[:], in_=xl_d.ap()[:, 0])
                    nc.sync.dma_start(out=aux0_sb[:], in_=aux_d.ap()[0])
            id_sb = constp.tile([128, 128], f16)
            nc.scalar.dma_start(out=id_sb[:], in_=id_d.ap()[:])
            ones_sb = constp.tile([1, 128], f16)
            nc.vector.memset(ones_sb[:], 1.0)
            bias_sb = constp.tile([1, TS], f16)
            nc.scalar.dma_start(out=bias_sb[:], in_=bias_d.ap()[:])

            def emit_tail(ubuf, adiag, Db, m):
                """Accumulation matmuls + final drain for batch chunk m.
                Emitted mid-way through chunk m+1 so the PE never waits on
                the coeff/Adiag chain."""
                out_ps = psO.tile([128, TS], f32, tag="out_ps")
                for it in range(IT):
                    nc.tensor.matmul(out_ps[:], adiag[:, it], ubuf[:, it],
                                     start=(it == 0), stop=False)
                # rank-1 column bias: ones^T @ bias_row
                nc.tensor.matmul(out_ps[:], ones_sb[:], bias_sb[:],
                                 start=False, stop=True)
                osb = outp.tile([128, TS], f32, tag="osb")
                nc.scalar.activation(osb[:], out_ps[:], Act.Identity,
                                     bias=Db[:], scale=1.0)
                nc.scalar.dma_start(out=out_d.ap()[m * 128:(m + 1) * 128, :],
                                    in_=osb[:])

            pending = None
            for m in range(MB):
                if m == 0:
                    xh_sb, xl_sb, aux_sb = xh0_sb, xl0_sb, aux0_sb
                else:
                    xh_sb = xmp.tile([128, IT * 2, 2, 128], f8, tag="xh")
                    nc.sync.dma_start(out=xh_sb[:], in_=xh_d.ap()[:, m])
                    xl_sb = xmp.tile([128, IT * 2, 2, 128], f8, tag="xl")
                    nc.sync.dma_start(out=xl_sb[:], in_=xl_d.ap()[:, m])
                    aux_sb = xmp.tile([128, 3 * IT], f32, tag="aux")
                    nc.sync.dma_start(out=aux_sb[:], in_=aux_d.ap()[m])

                ubuf = ubufp.tile([128, IT, TS], f16, tag="u16")
                pmaxb = statp.tile([128, IT], f32, tag="pmax")
                pminb = statp.tile([128, IT], f32, tag="pmin")
                umaxb = statp.tile([128, IT], f32, tag="umax")
                uminb = statp.tile([128, IT], f32, tag="umin")

                p_ps = None
                for it in range(IT):
                    # ---- P = x@L (paired PSUM tile: 2 tiles per 3D reduce) ----
                    if it % 2 == 0:
                        p_ps = psP.tile([128, 2, TS], f32, tag="p_ps")
                    ph = p_ps[:, it % 2]
                    nc.tensor.matmul(ph, xh_sb[:, 2 * it], l8_sb[:, 2 * it],
                                     start=True, stop=False, perf_mode=PM.DoubleRow)
                    nc.tensor.matmul(ph, xh_sb[:, 2 * it + 1],
                                     l8_sb[:, 2 * it + 1],
                                     start=False, stop=False, perf_mode=PM.DoubleRow)
                    nc.tensor.matmul(ph, xl_sb[:, 2 * it], l8_sb[:, 2 * it],
                                     start=False, stop=False, perf_mode=PM.DoubleRow)
                    nc.tensor.matmul(ph, xl_sb[:, 2 * it + 1],
                                     l8_sb[:, 2 * it + 1],
                                     start=False, stop=True, perf_mode=PM.DoubleRow)
                    # ---- u' = x@L - xh@(delta/step) ----
                    u_ps = psU.tile([128, TS], f32, tag="u_ps")
                    nc.tensor.matmul(u_ps[:], xh_sb[:, 2 * it], l8_sb[:, 2 * it],
                                     start=True, stop=False, perf_mode=PM.DoubleRow)
                    nc.tensor.matmul(u_ps[:], xh_sb[:, 2 * it + 1],
                                     l8_sb[:, 2 * it + 1],
                                     start=False, stop=False, perf_mode=PM.DoubleRow)
                    nc.tensor.matmul(u_ps[:], xl_sb[:, 2 * it], l8_sb[:, 2 * it],
                                     start=False, stop=False, perf_mode=PM.DoubleRow)
                    nc.tensor.matmul(u_ps[:], xl_sb[:, 2 * it + 1],
                                     l8_sb[:, 2 * it + 1],
                                     start=False, stop=False, perf_mode=PM.DoubleRow)
                    nc.tensor.matmul(u_ps[:], xh_sb[:, 2 * it], nd8_sb[:, 2 * it],
                                     start=False, stop=False, perf_mode=PM.DoubleRow)
                    nc.tensor.matmul(u_ps[:], xh_sb[:, 2 * it + 1],
                                     nd8_sb[:, 2 * it + 1],
                                     start=False, stop=False, perf_mode=PM.DoubleRow)
                    nc.tensor.matmul(u_ps[:], xh_sb[:, 2 * it], nd8l_sb[:, 2 * it],
                                     start=False, stop=False, perf_mode=PM.DoubleRow)
                    nc.tensor.matmul(u_ps[:], xh_sb[:, 2 * it + 1],
                                     nd8l_sb[:, 2 * it + 1],
                                     start=False, stop=True, perf_mode=PM.DoubleRow)

                    # drain u' -> fp16 SBUF (Act)
                    usl = ubuf[:, it]
                    nc.scalar.activation(usl, u_ps[:], Act.Identity,
                                         bias=0.0, scale=1.0)

                    # P range stats direct from PSUM (DVE), one 3D reduce
                    # per pair of tiles
                    if it % 2 == 1:
                        nc.vector.tensor_reduce(pmaxb[:, it - 1:it + 1], p_ps[:],
                                                axis=AX, op=Alu.max)
                        nc.vector.tensor_reduce(pminb[:, it - 1:it + 1], p_ps[:],
                                                axis=AX, op=Alu.min)

                    # u' range: paired 3D reduces over two tiles' fp16 SBUF
                    # copies (walrus rejects max/min TensorTensor on Pool, so
                    # DVE owns all range work)
                    if it % 2 == 1:
                        upair = ubuf[:, it - 1:it + 1]
                        nc.vector.tensor_reduce(umaxb[:, it - 1:it + 1], upair,
                                                axis=AX, op=Alu.max)
                        nc.vector.tensor_reduce(uminb[:, it - 1:it + 1], upair,
                                                axis=AX, op=Alu.min)

                    # previous chunk's accumulation, once this chunk's PE
                    # stream is deep enough to hide it
                    if it == 3 and pending is not None:
                        emit_tail(*pending)
                        pending = None

                # ---- per-m coefficient algebra on [128, IT] tiles (DVE) ----
                pr = statp.tile([128, IT], f32, tag="pr")
                ur = statp.tile([128, IT], f32, tag="ur")
                co = statp.tile([128, IT], f32, tag="co")
                Ab = statp.tile([128, IT], f32, tag="Ab")
                Rb = statp.tile([128, IT], f32, tag="Rb")
                Db = statp.tile([128, 1], f32, tag="Db")
                nc.vector.tensor_tensor(out=pr[:], in0=pmaxb[:], in1=pminb[:],
                                        op=Alu.subtract)
                nc.vector.tensor_tensor(out=ur[:], in0=umaxb[:], in1=uminb[:],
                                        op=Alu.subtract)
                nc.vector.reciprocal(out=ur[:], in_=ur[:])
                nc.vector.tensor_tensor(out=co[:], in0=pr[:], in1=ur[:],
                                        op=Alu.mult)
                # A = coeff * (step/s)_it ; aux cols [0:IT]
                nc.vector.tensor_tensor(out=Ab[:], in0=co[:],
                                        in1=aux_sb[:, 0:IT], op=Alu.mult)
                # R = E - A*umean ; umean cols [IT:2IT], E cols [2IT:3IT]
                nc.vector.tensor_tensor(out=Rb[:], in0=Ab[:],
                                        in1=aux_sb[:, IT:2 * IT], op=Alu.mult)
                nc.vector.tensor_tensor(out=Rb[:], in0=aux_sb[:, 2 * IT:3 * IT],
                                        in1=Rb[:], op=Alu.subtract)
                nc.vector.tensor_reduce(Db[:], Rb[:], axis=AX, op=Alu.add)

                # ---- Adiag build (Act); accumulation deferred to emit_tail ----
                adiag = adp.tile([128, IT, 128], f16, tag="adiag")
                for it in range(IT):
                    nc.scalar.activation(adiag[:, it], id_sb[:], Act.Identity,
                                         bias=0.0, scale=Ab[:, it:it + 1])
                pending = (ubuf, adiag, Db, m)
            emit_tail(*pending)

    nc.compile()
    return nc


def _host_prep(x, weight, bias):
    """Per-core input maps. Weight -> fp8 levels/deltas; x -> fp8 split;
    all row sums exact in f32/f64 on host."""
    import ml_dtypes
    f8 = ml_dtypes.float8_e4m3fn

    x = np.ascontiguousarray(x, dtype=np.float32)
    weight = np.ascontiguousarray(weight, dtype=np.float32)
    bias = np.ascontiguousarray(bias, dtype=np.float32)

    # ---- x: fp8 split, [k-part, m, it*2, pair, b] layout ----
    xt = x.T.astype(np.float64)                       # [4096, 1024]
    xh = xt.astype(f8)
    xl = (xt - xh.astype(np.float64)).astype(f8)
    # reshape rows (it, pg, pair, p) -> [p, m, it*pg, pair, b]

    def xlayout(a):
        a = a.reshape(IT, 2, 2, 128, MB, 128)          # t, g, c, p, m, b
        return np.ascontiguousarray(a.transpose(3, 4, 0, 1, 2, 5)
                                    .reshape(128, MB, IT * 2, 2, 128))

    xh_l = xlayout(xh)
    xl_l = xlayout(xl)

    xs = xt  # f64 exact for sums
    x8s = xh.astype(np.float64) + xl.astype(np.float64)   # device-x for sums? use exact
    xsum_it = xs.reshape(IT, TS, B).sum(axis=1)           # [IT, B]

    # r_wire
    i = np.arange(TS, dtype=np.float64)[:, None]
    j = np.arange(TS, dtype=np.float64)[None, :]
    r_wire = RP * ((TS - i) + (j + 1.0))
    step = float(STEP)

    ident = np.eye(128, dtype=np.float16)

    in_maps = []
    for d in range(NCORES):
        aux = np.empty((MB, 128, 3 * IT), dtype=np.float32)
        l8 = np.empty((IT, 2, 2, 128, TS), dtype=f8)
        nd8 = np.empty((IT, 2, 2, 128, TS), dtype=f8)
        nd8l = np.empty((IT, 2, 2, 128, TS), dtype=f8)
        for t in range(IT):
            w = weight[t * TS:(t + 1) * TS, d * TS:(d + 1) * TS].astype(np.float64)
            wmin, wmax = w.min(), w.max()
            s = (float(G_MAX) - float(G_MIN)) / (wmax - wmin + 1e-12)
            L = np.round((w - wmin) * s / step)
            q = L * step + float(G_MIN)
            g = 1.0 / (1.0 / q + r_wire)
            dd = (q - g) / step                      # delta/step
            L8 = L.astype(f8)
            ND8 = (-dd).astype(f8)
            ND8L = (-dd - ND8.astype(np.float64)).astype(f8)
            l8[t] = L8.reshape(2, 2, 128, TS)
            nd8[t] = ND8.reshape(2, 2, 128, TS)
            nd8l[t] = ND8L.reshape(2, 2, 128, TS)
            # sums with exact x
            xtile = xs[t * TS:(t + 1) * TS, :]       # [TS, B]
            Psum = L.sum(axis=1) @ xtile             # [B]
            Cdsum = dd.sum(axis=1) @ xtile
            usum = Psum - Cdsum
            umean = usum / TS
            imean = step * Psum / TS + float(G_MIN) * xsum_it[t]
            offset = xsum_it[t] * (float(G_MIN) - s * wmin)
            E = (imean - offset) / s
            aux[:, :, t] = (step / s) * np.ones((MB, 128), dtype=np.float32)
            aux[:, :, IT + t] = umean.reshape(MB, 128).astype(np.float32)
            aux[:, :, 2 * IT + t] = E.reshape(MB, 128).astype(np.float32)
        l8_l = np.ascontiguousarray(
            l8.transpose(3, 0, 1, 2, 4).reshape(128, IT * 2, 2, TS))
        nd8_l = np.ascontiguousarray(
            nd8.transpose(3, 0, 1, 2, 4).reshape(128, IT * 2, 2, TS))
        nd8l_l = np.ascontiguousarray(
            nd8l.transpose(3, 0, 1, 2, 4).reshape(128, IT * 2, 2, TS))
        in_maps.append({
            "xh": xh_l,
            "xl": xl_l,
            "l8": l8_l,
            "nd8": nd8_l,
            "nd8l": nd8l_l,
            "aux": aux,
            "ident": ident,
            "biasr": np.ascontiguousarray(
                bias[d * TS:(d + 1) * TS].astype(np.float16).reshape(1, TS)),
        })
    return in_maps


def get_nc():
    if "nc" not in _CACHE:
        _CACHE["nc"] = _build()
    return _CACHE["nc"]


def kernel(x, weight, bias):
    from concourse.bass_utils import run_bass_kernel_spmd

    nc = get_nc()
    in_maps = _host_prep(x, weight, bias)
    res = run_bass_kernel_spmd(nc, in_maps, core_ids=list(range(NCORES)))
    out = np.empty((B, OUT_F), dtype=np.float32)
    for d in range(NCORES):
        out[:, d * TS:(d + 1) * TS] = res.results[d]["out"]
    return out
